# revision 1
# baseline (speedup 1.0000x reference)
"""Trainium2 Bass kernel for the 8-layer LSTM + FC head model.

Strategy (see reference: B=4096, T=3, I=16, H=512, L=8):
  - Data-parallel over batch: 8 NeuronCores x 512 batch rows each,
    all ~16M parameters replicated per core.
  - On-chip state is kept transposed, hT[t] = h(t)^T with the hidden
    dim on SBUF partitions (4 chunks of 128) and batch (512) on the
    free dim.  Gate pre-activations are computed as
        gates^T[4H, B] = Wih^T-chunks @ x^T  +  Whh^T-chunks @ h^T
    accumulated in PSUM (f32), so the recurrence needs no per-step
    transposes; only the weights are pre-transposed (done host-side,
    together with a bf16 cast and the b_ih+b_hh fold).
  - ScalarE applies sigmoid/tanh with the per-partition gate bias fused
    (out = func(psum + bias)) directly from PSUM into SBUF f32.
  - VectorE does the cell update c = f*c + i*g (f32) and h = o*tanh(c)
    (bf16 out, feeding the next matmul).
  - t=0 skips the forget gate entirely (c(-1)=0) and all hidden-path
    matmuls.
  - FC head: y1^T[64,B] = fc1w^T-chunks @ hT(last layer, all t) + b1,
    then out[B,3] via lhsT=y1 (with a constant ones row folding in
    fc2_b) x rhs=fc2w^T.
"""

import sys

for _p in ("/root/.axon_site/_ro/trn_rl_repo", "/opt/trn_rl_repo"):
    if _p not in sys.path:
        sys.path.append(_p)

import numpy as np
import ml_dtypes

import concourse.bacc as bacc
import concourse.mybir as mybir
import concourse.tile as tile
from concourse.bass_utils import run_bass_kernel_spmd

B, T, I, H, L = 4096, 3, 16, 512, 8
N_CORES = 8
BL = B // N_CORES          # 512 batch rows per core
G4 = 4 * H                 # 2048 gate units
KC = H // 128              # 4 contraction chunks of 128
MC = G4 // 128             # 16 gate-unit chunks of 128
FK = (H * T) // 128        # 12 fc1 contraction chunks

BF16 = mybir.dt.bfloat16
F32 = mybir.dt.float32
Sigmoid = mybir.ActivationFunctionType.Sigmoid
Tanh = mybir.ActivationFunctionType.Tanh
Identity = mybir.ActivationFunctionType.Identity

_compiled = {}


def _build():
    nc = bacc.Bacc(
        "TRN2", target_bir_lowering=False, debug=False, num_devices=N_CORES
    )

    xT_d = nc.dram_tensor("xT", [16, T * BL], BF16, kind="ExternalInput")
    wih0T_d = nc.dram_tensor("wih0T", [16, G4], BF16, kind="ExternalInput")
    wihT_d = nc.dram_tensor("wihT", [L - 1, 128, KC * G4], BF16, kind="ExternalInput")
    whhT_d = nc.dram_tensor("whhT", [L, 128, KC * G4], BF16, kind="ExternalInput")
    bias_d = nc.dram_tensor("bias", [128, L * MC], F32, kind="ExternalInput")
    fc1wT_d = nc.dram_tensor("fc1wT", [128, FK * 64], BF16, kind="ExternalInput")
    fc1b_d = nc.dram_tensor("fc1b", [64, 1], F32, kind="ExternalInput")
    fc2wTb_d = nc.dram_tensor("fc2wTb", [65, 3], BF16, kind="ExternalInput")
    out_d = nc.dram_tensor("out", [BL, 3], F32, kind="ExternalOutput")

    with tile.TileContext(nc) as tc:
        with (
            tc.tile_pool(name="const", bufs=1) as constp,
            tc.tile_pool(name="wih", bufs=2) as wihp,
            tc.tile_pool(name="whh", bufs=2) as whhp,
            tc.tile_pool(name="hbuf", bufs=6) as hp,
            tc.tile_pool(name="cbuf", bufs=1) as cp,
            tc.tile_pool(name="gates", bufs=1) as gp,
            tc.tile_pool(name="elem", bufs=2) as ep,
            tc.tile_pool(name="fc", bufs=2) as fcp,
            tc.tile_pool(name="psum", bufs=6, space="PSUM") as pp,
            tc.tile_pool(name="psumfc", bufs=1, space="PSUM") as pfp,
        ):
            xT = constp.tile([16, T * BL], BF16, tag="xT", name="xT_s")
            nc.sync.dma_start(xT[:], xT_d[:])
            wih0T = constp.tile([16, G4], BF16, tag="wih0T", name="wih0T_s")
            nc.sync.dma_start(wih0T[:], wih0T_d[:])
            bias = constp.tile([128, L * MC], F32, tag="bias", name="bias_s")
            nc.sync.dma_start(bias[:], bias_d[:])
            fc1wT = constp.tile([128, FK * 64], BF16, tag="fc1wT", name="fc1wT_s")
            nc.sync.dma_start(fc1wT[:], fc1wT_d[:])
            fc1b = constp.tile([64, 1], F32, tag="fc1b", name="fc1b_s")
            nc.sync.dma_start(fc1b[:], fc1b_d[:])
            fc2wTb = constp.tile([65, 3], BF16, tag="fc2wTb", name="fc2wTb_s")
            nc.sync.dma_start(fc2wTb[:], fc2wTb_d[:])

            h_prev = None  # previous layer's hT tiles, one per t
            for l in range(L):
                if l > 0:
                    wih = wihp.tile([128, KC * G4], BF16, tag="wih", name=f"wih_{l}")
                    nc.sync.dma_start(wih[:], wihT_d[l - 1])
                whh = whhp.tile([128, KC * G4], BF16, tag="whh", name=f"whh_{l}")
                nc.sync.dma_start(whh[:], whhT_d[l])

                c = cp.tile([128, KC * BL], F32, tag="c", name=f"c_{l}")
                h_cur = []
                for t in range(T):
                    gates = gp.tile(
                        [128, MC * BL], F32, tag="gates", name=f"gates_{l}_{t}"
                    )
                    # Gate chunk order: put enough independent input-proj
                    # work ahead of the h(t-1)-dependent matmuls to cover
                    # the previous step's ACT/DVE tail.
                    for m in range(MC):
                        if t == 0 and 4 <= m < 8:
                            continue  # forget gate unused at t=0
                        ps = pp.tile([128, BL], F32, tag="ps", name=f"ps_{l}_{t}_{m}")
                        if l == 0:
                            nc.tensor.matmul(
                                ps[:],
                                wih0T[:, m * 128 : (m + 1) * 128],
                                xT[:, t * BL : (t + 1) * BL],
                                start=True,
                                stop=(t == 0),
                            )
                        else:
                            for k in range(KC):
                                nc.tensor.matmul(
                                    ps[:],
                                    wih[:, k * G4 + m * 128 : k * G4 + (m + 1) * 128],
                                    h_prev[t][:, k * BL : (k + 1) * BL],
                                    start=(k == 0),
                                    stop=(t == 0 and k == KC - 1),
                                )
                        if t > 0:
                            for k in range(KC):
                                nc.tensor.matmul(
                                    ps[:],
                                    whh[:, k * G4 + m * 128 : k * G4 + (m + 1) * 128],
                                    h_cur[t - 1][:, k * BL : (k + 1) * BL],
                                    start=False,
                                    stop=(k == KC - 1),
                                )
                        func = Tanh if 8 <= m < 12 else Sigmoid
                        nc.scalar.activation(
                            gates[:, m * BL : (m + 1) * BL],
                            ps[:],
                            func,
                            bias=bias[:, l * MC + m : l * MC + m + 1],
                        )

                    ht = hp.tile([128, KC * BL], BF16, tag="h", name=f"h_{l}_{t}")
                    for j in range(KC):
                        gi = gates[:, j * BL : (j + 1) * BL]
                        gf = gates[:, (j + 4) * BL : (j + 5) * BL]
                        gg = gates[:, (j + 8) * BL : (j + 9) * BL]
                        go = gates[:, (j + 12) * BL : (j + 13) * BL]
                        cj = c[:, j * BL : (j + 1) * BL]
                        if t == 0:
                            nc.vector.tensor_mul(cj, gi, gg)
                        else:
                            tmp = ep.tile(
                                [128, BL], F32, tag="tmp", name=f"tmp_{l}_{t}_{j}"
                            )
                            nc.vector.tensor_mul(tmp[:], gi, gg)
                            nc.vector.tensor_mul(cj, gf, cj)
                            nc.vector.tensor_add(cj, cj, tmp[:])
                        tanh_c = ep.tile(
                            [128, BL], F32, tag="tanhc", name=f"tanhc_{l}_{t}_{j}"
                        )
                        nc.scalar.activation(tanh_c[:], cj, Tanh)
                        nc.vector.tensor_mul(
                            ht[:, j * BL : (j + 1) * BL], go, tanh_c[:]
                        )
                    h_cur.append(ht)
                h_prev = h_cur

            # ---- FC head ----
            y1ps = pfp.tile([64, BL], F32, tag="y1", name="y1ps")
            for t in range(T):
                for j in range(KC):
                    kk = t * KC + j
                    nc.tensor.matmul(
                        y1ps[:],
                        fc1wT[:, kk * 64 : (kk + 1) * 64],
                        h_prev[t][:, j * BL : (j + 1) * BL],
                        start=(kk == 0),
                        stop=(kk == FK - 1),
                    )
            y1 = fcp.tile([65, BL], BF16, tag="y1s", name="y1s")
            nc.gpsimd.memset(y1[64:65, :], 1.0)
            nc.scalar.activation(y1[0:64, :], y1ps[:], Identity, bias=fc1b[:])

            for m in range(BL // 128):
                y2ps = pfp.tile([128, 3], F32, tag="y2", name=f"y2ps_{m}")
                nc.tensor.matmul(
                    y2ps[:],
                    y1[:, m * 128 : (m + 1) * 128],
                    fc2wTb[:],
                    start=True,
                    stop=True,
                )
                osb = fcp.tile([128, 3], F32, tag="osb", name=f"osb_{m}")
                nc.vector.tensor_copy(osb[:], y2ps[:])
                nc.sync.dma_start(out_d[m * 128 : (m + 1) * 128, :], osb[:])

    nc.compile()
    return nc


def _prep_inputs(inputs):
    """Host-side parameter repack: transposes / bf16 cast / bias fold."""
    bf = ml_dtypes.bfloat16
    x = np.asarray(inputs["x"], np.float32)            # [B, T, I]
    W_ih0 = np.asarray(inputs["W_ih0"], np.float32)    # [4H, I]
    W_ih_rest = np.asarray(inputs["W_ih_rest"], np.float32)  # [L-1, 4H, H]
    W_hh = np.asarray(inputs["W_hh"], np.float32)      # [L, 4H, H]
    b = (
        np.asarray(inputs["b_ih"], np.float32) + np.asarray(inputs["b_hh"], np.float32)
    )                                                  # [L, 4H]
    fc1_w = np.asarray(inputs["fc1_w"], np.float32)    # [64, H*T]
    fc1_b = np.asarray(inputs["fc1_b"], np.float32)    # [64]
    fc2_w = np.asarray(inputs["fc2_w"], np.float32)    # [3, 64]
    fc2_b = np.asarray(inputs["fc2_b"], np.float32)    # [3]

    def packT(w):  # [4H, D] -> [128, (D/128)*4H] bf16, W^T in k-chunk layout
        d = w.shape[1]
        kc = d // 128
        wt = np.ascontiguousarray(w.T)                  # [D, 4H]
        return np.ascontiguousarray(
            wt.reshape(kc, 128, G4).transpose(1, 0, 2).reshape(128, kc * G4)
        ).astype(bf)

    wih0T = np.ascontiguousarray(W_ih0.T).astype(bf)    # [16, 2048]
    wihT = np.stack([packT(W_ih_rest[i]) for i in range(L - 1)])  # [7,128,KC*G4]
    whhT = np.stack([packT(W_hh[i]) for i in range(L)])           # [8,128,KC*G4]
    # bias[p, l*16+m] = b[l, m*128+p]
    bias = np.ascontiguousarray(
        b.reshape(L, MC, 128).transpose(2, 0, 1).reshape(128, L * MC)
    )
    # fc1_w^T [H*T, 64] in k-chunk layout [128, FK*64]
    f1t = np.ascontiguousarray(fc1_w.T)                 # [1536, 64]
    fc1wT = np.ascontiguousarray(
        f1t.reshape(FK, 128, 64).transpose(1, 0, 2).reshape(128, FK * 64)
    ).astype(bf)
    fc1b = np.ascontiguousarray(fc1_b.reshape(64, 1))
    fc2wTb = np.concatenate(
        [np.ascontiguousarray(fc2_w.T), fc2_b.reshape(1, 3)], axis=0
    ).astype(bf)                                        # [65, 3]

    # x^T per core: [16, T*BL] with (t, b) on the free dim
    xt = np.ascontiguousarray(x.transpose(2, 1, 0)).astype(bf)  # [I, T, B]

    in_maps = []
    for ci in range(N_CORES):
        sl = slice(ci * BL, (ci + 1) * BL)
        in_maps.append(
            {
                "xT": np.ascontiguousarray(xt[:, :, sl]).reshape(16, T * BL),
                "wih0T": wih0T,
                "wihT": wihT,
                "whhT": whhT,
                "bias": bias,
                "fc1wT": fc1wT,
                "fc1b": fc1b,
                "fc2wTb": fc2wTb,
            }
        )
    return in_maps


def kernel(**inputs) -> np.ndarray:
    if "nc" not in _compiled:
        _compiled["nc"] = _build()
    nc = _compiled["nc"]
    in_maps = _prep_inputs(inputs)
    res = run_bass_kernel_spmd(nc, in_maps, core_ids=list(range(N_CORES)))
    out = np.concatenate([res.results[ci]["out"] for ci in range(N_CORES)], axis=0)
    return out.astype(np.float32)
